# revision 1
# baseline (speedup 1.0000x reference)
import sys
if '/opt/trn_rl_repo' not in sys.path:
    sys.path.insert(0, '/opt/trn_rl_repo')
import numpy as np

import concourse.bass as bass
import concourse.bacc as bacc
import concourse.tile as tile
from concourse import mybir
from concourse import bass_utils

f32 = mybir.dt.float32
f32r = mybir.dt.float32r
FX = mybir.ActivationFunctionType
ALU = mybir.AluOpType
AX = mybir.AxisListType

B, D, H, DH = 256, 256, 8, 32
NCORES = 8
BC = B // NCORES          # 32 batches per core
LC = 1024                 # self-attn KV cache length
NA = 2048                 # cross-attn key count
KT_S = LC // 128          # 8 key tiles (self)
KT_A = NA // 128          # 16 key tiles (cross)
SCALE = 1.0 / float(np.sqrt(DH))
EPS = 1e-5

WNAMES = ['wq_s', 'wk_s', 'wv_s', 'w0_s', 'wq_a', 'w0_a', 'w1', 'w2']
BNAMES = ['bq_s', 'bk_s', 'bv_s', 'b0_s', 'bq_a', 'b0_a', 'b1', 'b2']
LNAMES = ['ln1_g', 'ln1_b', 'ln2_g', 'ln2_b', 'ln3_g', 'ln3_b']


def _build():
    nc = bacc.Bacc()
    dr = {}
    dr['h_t'] = nc.dram_tensor('h_t', [BC, 1, D], f32, kind='ExternalInput')
    dr['K_att'] = nc.dram_tensor('K_att', [BC, NA, D], f32, kind='ExternalInput')
    dr['V_att'] = nc.dram_tensor('V_att', [BC, NA, D], f32r, kind='ExternalInput')
    dr['K_cache'] = nc.dram_tensor('K_cache', [BC, LC, D], f32, kind='ExternalInput')
    dr['V_cache'] = nc.dram_tensor('V_cache', [BC, LC, D], f32r, kind='ExternalInput')
    dr['notmT'] = nc.dram_tensor('notmT', [128, KT_A, BC], f32, kind='ExternalInput')
    dr['ident'] = nc.dram_tensor('ident', [128, 128], f32, kind='ExternalInput')
    dr['ones'] = nc.dram_tensor('ones', [128, 1], f32r, kind='ExternalInput')
    dr['zeros'] = nc.dram_tensor('zeros', [128, 2 * H * BC], f32r, kind='ExternalInput')
    for n in WNAMES:
        dr[n] = nc.dram_tensor(n, [D, D], f32r, kind='ExternalInput')
    for n in BNAMES + LNAMES:
        dr[n] = nc.dram_tensor(n, [D], f32, kind='ExternalInput')
    out = nc.dram_tensor('out', [BC, D], f32, kind='ExternalOutput')

    with tile.TileContext(nc) as tc:
        _emit(nc, tc, dr, out)
    nc.compile()
    return nc


def _emit(nc, tc, dr, out_dram):
    import contextlib
    ctx = contextlib.ExitStack()
    with ctx:
        const = ctx.enter_context(tc.tile_pool(name='const', bufs=1))
        keys_p = ctx.enter_context(tc.tile_pool(name='keys', bufs=3))
        vals_p = ctx.enter_context(tc.tile_pool(name='vals', bufs=3))
        ktl_p = ctx.enter_context(tc.tile_pool(name='ktl', bufs=2))
        kth_p = ctx.enter_context(tc.tile_pool(name='kth', bufs=2))
        wb_p = ctx.enter_context(tc.tile_pool(name='wb', bufs=2))
        wt_p = ctx.enter_context(tc.tile_pool(name='wt', bufs=2))
        sm_p = ctx.enter_context(tc.tile_pool(name='sm', bufs=4))
        tr_ps = ctx.enter_context(tc.tile_pool(name='trps', bufs=2, space='PSUM'))
        sc_ps = ctx.enter_context(tc.tile_pool(name='scps', bufs=2, space='PSUM'))
        at_ps = ctx.enter_context(tc.tile_pool(name='atps', bufs=2, space='PSUM'))
        ln_ps = ctx.enter_context(tc.tile_pool(name='lnps', bufs=1, space='PSUM'))
        gb_ps = ctx.enter_context(tc.tile_pool(name='gbps', bufs=1, space='PSUM'))

        garb = gb_ps.tile([1, 1], f32, tag='garb')
        last_act = [None]

        def pe_absorb(*aps):
            # PE matmul/transpose (fp32/fp32r self-loading weights) can carry only ONE
            # sem wait in its LW slot. Before a matmul whose deps span several procs,
            # emit 1x1 self-matmuls so the PE observes those sems here instead.
            for a in aps:
                if a is None:
                    continue
                e = a[tuple(slice(0, 1) for _ in range(len(a.shape)))]
                if e.dtype == f32r:
                    e = e.bitcast(f32)
                nc.tensor.matmul(garb[:, :], e, e, start=True, stop=True,
                                 skip_group_check=True)

        # ---------- persistent loads ----------
        ident = const.tile([128, 128], f32, tag='ident')
        nc.sync.dma_start(out=ident, in_=dr['ident'][:, :])
        pe_absorb(ident)
        ones = const.tile([128, 1], f32r, tag='ones')
        nc.sync.dma_start(out=ones, in_=dr['ones'][:, :])
        epst = const.tile([BC, 1], f32, tag='epst')
        nc.vector.memset(epst, EPS)

        wsb = {}
        for n in WNAMES:
            wsb[n] = const.tile([128, 2, D], f32r, tag='w_' + n, name='w_' + n)
            nc.sync.dma_start(out=wsb[n], in_=dr[n][:, :].rearrange('(t p) j -> p t j', p=128))
        vsb = {}
        for n in BNAMES + LNAMES:
            vsb[n] = const.tile([BC, D], f32, tag='v_' + n, name='v_' + n)
            nc.gpsimd.dma_start(out=vsb[n], in_=dr[n][:].unsqueeze(0).to_broadcast([BC, D]))

        notmT = const.tile([128, KT_A, BC], f32, tag='notmT')
        nc.sync.dma_start(out=notmT, in_=dr['notmT'][:, :, :])

        ht = const.tile([BC, D], f32, tag='ht')
        nc.sync.dma_start(out=ht, in_=dr['h_t'][:, 0, :])
        pe_absorb(ht)

        # ---------- helpers ----------
        def transpose_128(dst, src, cols):
            # src [rows<=128, cols<=128] SBUF f32 -> dst [cols, rows] via PE transpose
            rows = src.shape[0]
            ps = tr_ps.tile([128, 128], f32, tag='trps')
            nc.tensor.transpose(ps[0:cols, 0:rows], src, ident[0:rows, 0:rows])
            nc.vector.tensor_copy(out=dst, in_=ps[0:cols, 0:rows])

        def make_T(src_f32, tagname):
            # src [BC, D] -> [128, 2, BC] f32r transposed halves
            dstT = const.tile([128, 2, BC], f32r, tag=tagname, name=tagname)
            for t in range(2):
                transpose_128(dstT[:, t, :], src_f32[:, 128 * t:128 * (t + 1)], 128)
            return dstT

        def linear_psum(srcT_list, wname):
            # sum_t sum_s srcT.T @ W  -> psum [BC, D]
            ps = ln_ps.tile([BC, D], f32, tag='lnps')
            pe_absorb(wsb[wname])
            n_mm = 2 * len(srcT_list)
            i = 0
            for srcT in srcT_list:
                for t in range(2):
                    nc.tensor.matmul(ps[:, :], srcT[:, t, :], wsb[wname][:, t, :],
                                     start=(i == 0), stop=(i == n_mm - 1))
                    i += 1
            return ps

        def layernorm(dst, src, gname, bname, tagp):
            stats = const.tile([BC, 6], f32, tag=tagp + '_st', name=tagp + '_st')
            nc.vector.bn_stats(out=stats, in_=src)
            mv = const.tile([BC, 2], f32, tag=tagp + '_mv', name=tagp + '_mv')
            nc.vector.bn_aggr(out=mv, in_=stats)
            sd = const.tile([BC, 1], f32, tag=tagp + '_sd', name=tagp + '_sd')
            nc.scalar.activation(out=sd, in_=mv[:, 1:2], func=FX.Sqrt,
                                 bias=epst[:, :], scale=1.0)
            rstd = const.tile([BC, 1], f32, tag=tagp + '_rs', name=tagp + '_rs')
            nc.vector.reciprocal(out=rstd, in_=sd)
            nc.vector.tensor_scalar(out=dst, in0=src, scalar1=mv[:, 0:1], scalar2=rstd,
                                    op0=ALU.subtract, op1=ALU.mult)
            nc.vector.tensor_mul(dst, dst, vsb[gname])
            nc.vector.tensor_add(dst, dst, vsb[bname])

        def build_qblk(qsrc_f32, tagp):
            qT = make_T(qsrc_f32, tagp + '_qT')
            qb = const.tile([128, 2, H, BC], f32r, tag=tagp + '_qb', name=tagp + '_qb')
            nc.sync.dma_start(out=qb, in_=dr['zeros'][:, :].rearrange('p (t h b) -> p t h b', t=2, h=H))
            pe_absorb(qb)
            for t in range(2):
                for hh in range(4):
                    h = 4 * t + hh
                    nc.vector.tensor_copy(out=qb[32 * hh:32 * (hh + 1), t, h, :],
                                          in_=qT[32 * hh:32 * (hh + 1), t, :])
            return qb

        # ---------- qkv for self-attn ----------
        htT = make_T(ht, 'htT')
        qkv = {}
        for nm, wn, bn in (('q', 'wq_s', 'bq_s'), ('k', 'wk_s', 'bk_s'), ('v', 'wv_s', 'bv_s')):
            ps = linear_psum([htT], wn)
            qkv[nm] = const.tile([BC, D], f32, tag='qkv_' + nm, name='qkv_' + nm)
            nc.vector.tensor_add(qkv[nm], ps, vsb[bn])

        qblk_s = build_qblk(qkv['q'], 'self')

        # new-key (appended k/v) terms, all-batch
        qk = const.tile([BC, D], f32, tag='qk')
        nc.vector.tensor_mul(qk, qkv['q'], qkv['k'])
        s_new = const.tile([BC, H], f32, tag='s_new')
        nc.vector.reduce_sum(out=s_new, in_=qk.rearrange('p (g s) -> p g s', g=H), axis=AX.X)
        w_new = const.tile([BC, H], f32, tag='w_new')
        nc.scalar.activation(out=w_new, in_=s_new, func=FX.Exp, scale=SCALE)
        w_newT = const.tile([H, BC], f32, tag='w_newT')
        pe_absorb(w_new)
        transpose_128(w_newT, w_new, H)

        invmix = const.tile([H, BC], f32, tag='invmix')

        # ---------- attention inner loop ----------
        def attention(qblk, n_tiles, K_dram, V_dram, attT_dst, masked, inv_store):
            for b in range(BC):
                kc = keys_p.tile([128, KT_A, D], f32, tag='keys')
                nc.sync.dma_start(out=kc[:, 0:n_tiles, :],
                                  in_=K_dram[b].rearrange('(t p) d -> p t d', p=128))
                vc = vals_p.tile([128, KT_A, D + 4], f32r, tag='vals')
                nc.sync.dma_start(out=vc[:, 0:n_tiles, 0:D],
                                  in_=V_dram[b].rearrange('(t p) d -> p t d', p=128))
                nc.vector.tensor_copy(out=vc[:, 0:n_tiles, D:D + 4],
                                      in_=ones.unsqueeze(1).broadcast_to([128, n_tiles, 4]))
                pe_absorb(kc, vc)
                ktl = ktl_p.tile([128, KT_A, 128], f32r, tag='ktl')
                kth = kth_p.tile([128, KT_A, 128], f32r, tag='kth')
                for t in range(n_tiles):
                    ps1 = tr_ps.tile([128, 128], f32, tag='trps')
                    nc.tensor.transpose(ps1[:, :], kc[:, t, 0:128], ident)
                    nc.vector.tensor_copy(out=ktl[:, t, :], in_=ps1)
                    ps2 = tr_ps.tile([128, 128], f32, tag='trps')
                    nc.tensor.transpose(ps2[:, :], kc[:, t, 128:256], ident)
                    nc.vector.tensor_copy(out=kth[:, t, :], in_=ps2)
                wb = wb_p.tile([H, KT_A * 128], f32, tag='wb')
                for c in range(n_tiles // 4):
                    ssp = sc_ps.tile([H, 512], f32, tag='scps')
                    if last_act[0] is not None:
                        pe_absorb(last_act[0])
                    nc.tensor.matmul(ssp[:, :], qblk[:, 0, :, b], ktl[:, 4 * c:4 * (c + 1), :],
                                     start=True, stop=False)
                    nc.tensor.matmul(ssp[:, :], qblk[:, 1, :, b], kth[:, 4 * c:4 * (c + 1), :],
                                     start=False, stop=True)
                    nc.scalar.activation(out=wb[:, 512 * c:512 * (c + 1)], in_=ssp,
                                         func=FX.Exp, scale=SCALE)
                    last_act[0] = wb[:, 512 * c:512 * (c + 1)]
                atp = at_ps.tile([H, D + 4], f32, tag='atps')
                wtt = wt_p.tile([128, KT_A, H], f32r, tag='wt')
                for t in range(n_tiles):
                    if t % 4 == 0:
                        pe_absorb(wb[:, 512 * (t // 4):512 * (t // 4) + 1])
                    pw = tr_ps.tile([128, 128], f32, tag='trps')
                    nc.tensor.transpose(pw[0:128, 0:H], wb[:, 128 * t:128 * (t + 1)],
                                        ident[0:H, 0:H])
                    if masked:
                        nc.vector.tensor_scalar_mul(out=wtt[:, t, :], in0=pw[:, 0:H],
                                                    scalar1=notmT[:, t, b:b + 1])
                    else:
                        nc.vector.tensor_copy(out=wtt[:, t, :], in_=pw[:, 0:H])
                    nc.tensor.matmul(atp[:, :], wtt[:, t, :], vc[:, t, :],
                                     start=(t == 0), stop=(t == n_tiles - 1),
                                     skip_group_check=True)
                # denominator -> inverse
                dn = sm_p.tile([H, 1], f32, tag='dn')
                if inv_store is not None:
                    nc.vector.tensor_add(dn, atp[:, D:D + 1], w_newT[:, b:b + 1])
                else:
                    nc.vector.tensor_copy(out=dn, in_=atp[:, D:D + 1])
                iv = sm_p.tile([H, 1], f32, tag='iv')
                nc.vector.reciprocal(out=iv, in_=dn)
                if inv_store is not None:
                    nc.vector.tensor_copy(out=inv_store[:, b:b + 1], in_=iv)
                # scaled mixed attention, then un-mix via transpose + 32-aligned copies
                attm = sm_p.tile([H, D], f32, tag='attm')
                nc.vector.tensor_scalar_mul(out=attm, in0=atp[:, 0:D], scalar1=iv)
                for t in range(2):
                    pa = tr_ps.tile([128, 128], f32, tag='trps')
                    nc.tensor.transpose(pa[0:128, 0:H], attm[:, 128 * t:128 * (t + 1)],
                                        ident[0:H, 0:H])
                    for k in range(4):
                        h = 4 * t + k
                        nc.vector.tensor_copy(out=attT_dst[32 * k:32 * (k + 1), t, b:b + 1],
                                              in_=pa[32 * k:32 * (k + 1), h:h + 1])

        # ---------- self attention ----------
        attT_s = const.tile([128, 2, BC], f32r, tag='attT_s')
        attention(qblk_s, KT_S, dr['K_cache'], dr['V_cache'], attT_s, False, invmix)

        # new-key numerator: nv = v * w_new * inv  (batch layout), then transpose
        invb = const.tile([BC, H], f32, tag='invb')
        transpose_128(invb, invmix, BC)
        nv = const.tile([BC, D], f32, tag='nv')
        nc.vector.tensor_tensor(out=nv.rearrange('p (g s) -> p g s', g=H),
                                in0=qkv['v'].rearrange('p (g s) -> p g s', g=H),
                                in1=w_new.unsqueeze(2).broadcast_to([BC, H, DH]),
                                op=ALU.mult)
        nc.vector.tensor_tensor(out=nv.rearrange('p (g s) -> p g s', g=H),
                                in0=nv.rearrange('p (g s) -> p g s', g=H),
                                in1=invb.unsqueeze(2).broadcast_to([BC, H, DH]),
                                op=ALU.mult)
        nvT = make_T(nv, 'nvT')

        # h1 = LN1(ht + att_self @ w0_s + b0_s)
        ps = linear_psum([attT_s, nvT], 'w0_s')
        h1p = const.tile([BC, D], f32, tag='h1p')
        nc.vector.tensor_add(h1p, ps, vsb['b0_s'])
        nc.vector.tensor_add(h1p, h1p, ht)
        h1 = const.tile([BC, D], f32, tag='h1')
        layernorm(h1, h1p, 'ln1_g', 'ln1_b', 'ln1')

        # ---------- cross attention ----------
        h1T = make_T(h1, 'h1T')
        psq = linear_psum([h1T], 'wq_a')
        qa = const.tile([BC, D], f32, tag='qa')
        nc.vector.tensor_add(qa, psq, vsb['bq_a'])
        qblk_a = build_qblk(qa, 'cross')

        attT_a = const.tile([128, 2, BC], f32r, tag='attT_a')
        attention(qblk_a, KT_A, dr['K_att'], dr['V_att'], attT_a, True, None)

        # h2 = LN2(h1 + att_cross @ w0_a + b0_a)
        ps2 = linear_psum([attT_a], 'w0_a')
        h2p = const.tile([BC, D], f32, tag='h2p')
        nc.vector.tensor_add(h2p, ps2, vsb['b0_a'])
        nc.vector.tensor_add(h2p, h2p, h1)
        h2 = const.tile([BC, D], f32, tag='h2')
        layernorm(h2, h2p, 'ln2_g', 'ln2_b', 'ln2')

        # ---------- MLP ----------
        h2T = make_T(h2, 'h2T')
        psm = linear_psum([h2T], 'w1')
        m1 = const.tile([BC, D], f32, tag='m1')
        nc.vector.tensor_add(m1, psm, vsb['b1'])
        m1r = const.tile([BC, D], f32, tag='m1r')
        nc.scalar.activation(out=m1r, in_=m1, func=FX.Relu, scale=1.0)
        pe_absorb(m1r)
        m1T = make_T(m1r, 'm1T')
        psm2 = linear_psum([m1T], 'w2')
        h3p = const.tile([BC, D], f32, tag='h3p')
        nc.vector.tensor_add(h3p, psm2, vsb['b2'])
        nc.vector.tensor_add(h3p, h3p, h2)
        outt = const.tile([BC, D], f32, tag='outt')
        layernorm(outt, h3p, 'ln3_g', 'ln3_b', 'ln3')
        nc.sync.dma_start(out=out_dram[:, :], in_=outt)


_CACHE = {}


def _get_nc():
    if 'nc' not in _CACHE:
        _CACHE['nc'] = _build()
    return _CACHE['nc']


def _make_in_maps(inputs):
    np_in = {k: np.ascontiguousarray(np.asarray(v)) for k, v in inputs.items()}
    ident = np.eye(128, dtype=np.float32)
    ones = np.ones((128, 1), dtype=np.float32)
    zeros = np.zeros((128, 2 * H * BC), dtype=np.float32)
    in_maps = []
    for c in range(NCORES):
        sl = slice(c * BC, (c + 1) * BC)
        m = np_in['mask'][sl].astype(np.float32)          # [BC, NA], True = masked
        notm = (1.0 - m).reshape(BC, KT_A, 128).transpose(2, 1, 0).copy()  # [128, KT_A, BC]
        im = {
            'h_t': np_in['h_t'][sl],
            'K_att': np_in['K_att'][sl],
            'V_att': np_in['V_att'][sl],
            'K_cache': np_in['K_cache'][sl],
            'V_cache': np_in['V_cache'][sl],
            'notmT': notm,
            'ident': ident,
            'ones': ones,
            'zeros': zeros,
        }
        for n in WNAMES + BNAMES + LNAMES:
            im[n] = np_in[n]
        in_maps.append(im)
    return in_maps


def run_on_device(inputs):
    nc = _get_nc()
    in_maps = _make_in_maps(inputs)
    res = bass_utils.run_bass_kernel_spmd(nc, in_maps, core_ids=list(range(NCORES)),
                                          trace=False)
    outs = [res.results[c]['out'] for c in range(NCORES)]
    return np.concatenate(outs, axis=0).astype(np.float32)


def kernel(**inputs):
    return run_on_device(inputs)



# revision 17
# speedup vs baseline: 53.3529x; 53.3529x over previous
import sys
if '/opt/trn_rl_repo' not in sys.path:
    sys.path.insert(0, '/opt/trn_rl_repo')
import numpy as np

import concourse.bass as bass
import concourse.bacc as bacc
import concourse.tile as tile
from concourse import mybir
from concourse import bass_utils

f32 = mybir.dt.float32
f32r = mybir.dt.float32r
bf16 = mybir.dt.bfloat16
FX = mybir.ActivationFunctionType
ALU = mybir.AluOpType
AX = mybir.AxisListType

B, D, H, DH = 256, 256, 8, 32
NCORES = 8
BC = B // NCORES          # 32 batches per core
LC = 1024                 # self-attn KV cache length
NA = 2048                 # cross-attn key count
KT_S = LC // 128          # 8 key tiles (self)
KT_A = NA // 128          # 16 key tiles (cross)
SCALE = 1.0 / float(np.sqrt(DH))
EPS = 1e-5
QLOOK = 3                 # q-broadcast prefetch distance (batches)
SELF_MULT_POOL = False    # gpsimd scalar_tensor_tensor is not compilable by walrus

WNAMES = ['wq_s', 'wk_s', 'wv_s', 'w0_s', 'wq_a', 'w0_a', 'w1', 'w2']
BNAMES = ['bq_s', 'bk_s', 'bv_s', 'b0_s', 'bq_a', 'b0_a', 'b1', 'b2']
LNAMES = ['ln1_g', 'ln1_b', 'ln2_g', 'ln2_b', 'ln3_g', 'ln3_b']


def _r(ap):
    return ap if ap.dtype == f32r else ap.bitcast(f32r)


def _build(repeat=1):
    nc = bacc.Bacc()
    dr = {}
    dr['h_t'] = nc.dram_tensor('h_t', [BC, 1, D], f32, kind='ExternalInput')
    dr['K_att'] = nc.dram_tensor('K_att', [BC, NA, D], f32, kind='ExternalInput')
    dr['V_att'] = nc.dram_tensor('V_att', [BC, NA, D], f32r, kind='ExternalInput')
    dr['K_cache'] = nc.dram_tensor('K_cache', [BC, LC, D], f32, kind='ExternalInput')
    dr['V_cache'] = nc.dram_tensor('V_cache', [BC, LC, D], f32r, kind='ExternalInput')
    dr['notmT'] = nc.dram_tensor('notmT', [128, KT_A, BC], f32, kind='ExternalInput')
    dr['ident'] = nc.dram_tensor('ident', [128, 128], f32, kind='ExternalInput')
    dr['identr'] = nc.dram_tensor('identr', [128, 128], f32r, kind='ExternalInput')
    dr['onesr'] = nc.dram_tensor('onesr', [128, 1], f32r, kind='ExternalInput')
    for n in WNAMES:
        dr[n] = nc.dram_tensor(n, [D, D], f32r, kind='ExternalInput')
    for n in BNAMES + LNAMES:
        dr[n] = nc.dram_tensor(n, [D], f32, kind='ExternalInput')
    out = nc.dram_tensor('out', [BC, D], f32, kind='ExternalOutput')

    with tile.TileContext(nc) as tc:
        for _ in range(repeat):
            _emit(nc, tc, dr, out)
    nc.compile()
    return nc


def _emit(nc, tc, dr, out_dram):
    import contextlib
    ctx = contextlib.ExitStack()
    with ctx:
        const = ctx.enter_context(tc.tile_pool(name='const', bufs=1))
        ck_p = ctx.enter_context(tc.tile_pool(name='ck', bufs=3))
        cv_p = ctx.enter_context(tc.tile_pool(name='cv', bufs=3))
        sk_p = ctx.enter_context(tc.tile_pool(name='sk', bufs=2))
        sv_p = ctx.enter_context(tc.tile_pool(name='sv', bufs=2))
        qb_p = ctx.enter_context(tc.tile_pool(name='qb', bufs=QLOOK + 2))
        pr_p = ctx.enter_context(tc.tile_pool(name='pr', bufs=2))
        sc_p = ctx.enter_context(tc.tile_pool(name='sc', bufs=2))
        w_p = ctx.enter_context(tc.tile_pool(name='w', bufs=2))
        wm_p = ctx.enter_context(tc.tile_pool(name='wm', bufs=2))
        ws_p = ctx.enter_context(tc.tile_pool(name='ws', bufs=2))
        dn_sb = ctx.enter_context(tc.tile_pool(name='dnsb', bufs=2))
        iv_sb = ctx.enter_context(tc.tile_pool(name='ivsb', bufs=2))
        at_sb = ctx.enter_context(tc.tile_pool(name='atsb', bufs=3))
        tr_ps = ctx.enter_context(tc.tile_pool(name='trps', bufs=1, space='PSUM'))
        aux_ps = ctx.enter_context(tc.tile_pool(name='auxps', bufs=1, space='PSUM'))
        at_ps = ctx.enter_context(tc.tile_pool(name='atps', bufs=3, space='PSUM'))
        dn_ps = ctx.enter_context(tc.tile_pool(name='dnps', bufs=2, space='PSUM'))
        bc_ps = aux_ps
        ln_ps = aux_ps

        garb = tr_ps.tile([1, 1], f32, tag='garb')

        def pe_absorb(*aps):
            # PE matmul/transpose can carry only ONE sem wait in its LW slot.
            # Before a matmul whose deps span several producers, emit 1x1
            # self-matmuls so the PE observes those sems here instead.
            for a in aps:
                if a is None:
                    continue
                e = a[tuple(slice(0, 1) for _ in range(len(a.shape)))]
                if e.dtype == f32r:
                    e = e.bitcast(f32)
                nc.tensor.matmul(garb[:, :], e, e, start=True, stop=True,
                                 skip_group_check=True)

        # ---------- persistent loads ----------
        ident = const.tile([128, 128], f32, tag='ident')
        nc.sync.dma_start(out=ident, in_=dr['ident'][:, :])
        pe_absorb(ident)
        identr = const.tile([128, 128], f32r, tag='identr')
        nc.sync.dma_start(out=identr, in_=dr['identr'][:, :])
        onesr = const.tile([128, 1], f32r, tag='onesr')
        nc.sync.dma_start(out=onesr, in_=dr['onesr'][:, :])
        epst = const.tile([BC, 1], f32, tag='epst')
        nc.vector.memset(epst, EPS)

        wsb = {}
        for n in WNAMES:
            wsb[n] = const.tile([128, 2, D], f32r, tag='w_' + n, name='w_' + n)
            nc.sync.dma_start(out=wsb[n], in_=dr[n][:, :].rearrange('(t p) j -> p t j', p=128))
        vsb = {}
        for n in BNAMES + LNAMES:
            vsb[n] = const.tile([BC, D], f32, tag='v_' + n, name='v_' + n)
            nc.gpsimd.dma_start(out=vsb[n], in_=dr[n][:].unsqueeze(0).to_broadcast([BC, D]))

        notmT = const.tile([128, KT_A, BC], f32, tag='notmT')
        nc.sync.dma_start(out=notmT, in_=dr['notmT'][:, :, :])

        ht = const.tile([BC, D], f32, tag='ht')
        nc.sync.dma_start(out=ht, in_=dr['h_t'][:, 0, :])
        pe_absorb(ht)

        # ---------- helpers ----------
        def transpose_128(dst, src, cols):
            rows = src.shape[0]
            ps = tr_ps.tile([128, 128], f32, tag='trps')
            nc.tensor.transpose(ps[0:cols, 0:rows], src, ident[0:rows, 0:rows])
            nc.vector.tensor_copy(out=dst, in_=ps[0:cols, 0:rows])

        def make_T(src_f32, tagname):
            dstT = const.tile([128, 2, BC], f32r, tag=tagname, name=tagname)
            for t in range(2):
                transpose_128(dstT[:, t, :], src_f32[:, 128 * t:128 * (t + 1)], 128)
            return dstT

        def linear_psum(srcT_list, wname):
            ps = ln_ps.tile([BC, D], f32, tag='aux')
            pe_absorb(wsb[wname])
            n_mm = 2 * len(srcT_list)
            i = 0
            for srcT in srcT_list:
                for t in range(2):
                    nc.tensor.matmul(ps[:, :], _r(srcT[:, t, :]), wsb[wname][:, t, :],
                                     start=(i == 0), stop=(i == n_mm - 1))
                    i += 1
            return ps

        def layernorm(dst, src, gname, bname, tagp):
            stats = const.tile([BC, 6], f32, tag=tagp + '_st', name=tagp + '_st')
            nc.vector.bn_stats(out=stats, in_=src)
            mv = const.tile([BC, 2], f32, tag=tagp + '_mv', name=tagp + '_mv')
            nc.vector.bn_aggr(out=mv, in_=stats)
            sd = const.tile([BC, 1], f32, tag=tagp + '_sd', name=tagp + '_sd')
            nc.scalar.activation(out=sd, in_=mv[:, 1:2], func=FX.Sqrt,
                                 bias=epst[:, :], scale=1.0)
            rstd = const.tile([BC, 1], f32, tag=tagp + '_rs', name=tagp + '_rs')
            nc.vector.reciprocal(out=rstd, in_=sd)
            nc.vector.tensor_scalar(out=dst, in0=src, scalar1=mv[:, 0:1], scalar2=rstd,
                                    op0=ALU.subtract, op1=ALU.mult)
            nc.vector.tensor_mul(dst, dst, vsb[gname])
            nc.vector.tensor_add(dst, dst, vsb[bname])

        def build_qb(q_sb, b):
            # qb[b] = q_sb[b, :] replicated across all 128 partitions, via a
            # selector matmul: lhsT = e_b (col b of identity) broadcast along
            # the free axis, so out[p, d] = sum_c delta(c, b) q[c, d] = q[b, d].
            ps = bc_ps.tile([128, D], f32, tag='aux')
            sel = identr[0:BC, b:b + 1].broadcast_to([BC, 128])
            nc.tensor.matmul(ps[:, :], sel, q_sb, start=True, stop=True,
                             skip_group_check=True)
            qb = qb_p.tile([128, D], f32, tag='qb')
            nc.scalar.copy(out=qb, in_=ps[:, :])
            return qb

        # ---------- qkv for self-attn ----------
        htT = make_T(ht, 'htT')
        qkv = {}
        for nm, wn, bn in (('q', 'wq_s', 'bq_s'), ('k', 'wk_s', 'bk_s'), ('v', 'wv_s', 'bv_s')):
            ps = linear_psum([htT], wn)
            dt = f32r if nm == 'q' else f32
            qkv[nm] = const.tile([BC, D], dt, tag='qkv_' + nm, name='qkv_' + nm)
            nc.vector.tensor_add(qkv[nm], ps, vsb[bn])

        # new-key (appended k/v) terms, all-batch
        qk = const.tile([BC, D], f32, tag='qk')
        nc.vector.tensor_mul(qk, qkv['q'], qkv['k'])
        s_new = const.tile([BC, H], f32, tag='s_new')
        nc.vector.reduce_sum(out=s_new, in_=qk.rearrange('p (g s) -> p g s', g=H), axis=AX.X)
        w_new = const.tile([BC, H], f32r, tag='w_new')
        nc.scalar.activation(out=w_new, in_=s_new, func=FX.Exp, scale=SCALE)
        pe_absorb(w_new)

        # full self-attn denominators (incl. w_new), one row per batch on
        # partition 0, collected during the self loop for the nv term
        dnst = const.tile([1, BC, H], f32, tag='dnst')

        # ---------- attention inner loop ----------
        # The softmax denominator is computed with an all-ones matmul over the
        # weights (partition sums replicated on every partition), and 1/den is
        # multiplied into the weights BEFORE the w@V matmul, so its PSUM output
        # is already normalized. Nothing in the steady-state loop reads the
        # attention PSUM except one ACT copy deferred by two batches; all other
        # waits are short-range, so no engine wait-queue ever parks long.
        def attention(n_tiles, K_dram, V_dram, q_sb, attT_dst, masked, self_newkey,
                      kp, vp, pool_mult):
            pe_absorb(q_sb)
            qbs = {b: build_qb(q_sb, b) for b in range(min(QLOOK, BC))}

            def epilogue(atp, b):
                attm = at_sb.tile([H, D], f32, tag='attm')
                nc.scalar.copy(out=attm, in_=atp[:, :])
                for t in range(2):
                    pa = tr_ps.tile([128, 128], f32, tag='trps')
                    nc.tensor.transpose(pa[0:128, 0:H], attm[:, 128 * t:128 * (t + 1)],
                                        ident[0:H, 0:H])
                    for k in range(4):
                        h = 4 * t + k
                        nc.scalar.copy(out=attT_dst[32 * k:32 * (k + 1), t, b:b + 1],
                                       in_=pa[32 * k:32 * (k + 1), h:h + 1])

            pend = []
            for b in range(BC):
                if b + QLOOK < BC:
                    qbs[b + QLOOK] = build_qb(q_sb, b + QLOOK)
                kc = kp.tile([128, n_tiles, D], f32, tag='kc')
                nc.sync.dma_start(out=kc,
                                  in_=K_dram[b].rearrange('(t p) d -> p t d', p=128))
                vc = vp.tile([128, n_tiles, D], f32r, tag='vc')
                nc.sync.dma_start(out=vc,
                                  in_=V_dram[b].rearrange('(t p) d -> p t d', p=128))
                qb = qbs.pop(b)
                # scores^T[k, t, h] = sum_s K[k, t, h*32+s] * q[h*32+s] * SCALE
                prod = pr_p.tile([128, n_tiles, D], bf16, tag='prod')
                qb_b = qb.unsqueeze(1).broadcast_to([128, n_tiles, D])
                if pool_mult:
                    nc.gpsimd.scalar_tensor_tensor(out=prod, in0=kc, scalar=SCALE,
                                                   in1=qb_b, op0=ALU.mult, op1=ALU.mult)
                else:
                    nc.vector.scalar_tensor_tensor(out=prod, in0=kc, scalar=SCALE,
                                                   in1=qb_b, op0=ALU.mult, op1=ALU.mult)
                sc = sc_p.tile([128, n_tiles, H], f32, tag='sc')
                nc.vector.tensor_reduce(out=sc,
                                        in_=prod.rearrange('p t (h s) -> p t h s', h=H),
                                        axis=AX.X, op=ALU.add)
                w = w_p.tile([128, n_tiles, H], f32r, tag='w')
                nc.scalar.activation(out=w, in_=sc, func=FX.Exp, scale=1.0)
                if masked:
                    wm = wm_p.tile([128, n_tiles, H], f32r, tag='wm')
                    nc.vector.tensor_tensor(
                        out=wm, in0=w,
                        in1=notmT[:, 0:n_tiles, b:b + 1].broadcast_to([128, n_tiles, H]),
                        op=ALU.mult)
                else:
                    wm = w
                # denominator: all-ones matmul -> per-(t,h) partition sums on
                # every partition, laid out h-major so the t-reduce is innermost
                dps = dn_ps.tile([128, H, n_tiles], f32, tag='dn')
                nc.tensor.matmul(dps[:, :, :],
                                 onesr.broadcast_to([128, 128]),
                                 wm.rearrange('p t h -> p h t'),
                                 start=True, stop=True, skip_group_check=True)
                dnb = dn_sb.tile([128, H], f32, tag='dnb')
                nc.vector.tensor_reduce(out=dnb, in_=dps, axis=AX.X, op=ALU.add)
                if self_newkey:
                    wnps = bc_ps.tile([128, H], f32, tag='aux')
                    seln = identr[0:BC, b:b + 1].broadcast_to([BC, 128])
                    nc.tensor.matmul(wnps[:, :], seln, w_new, start=True,
                                     stop=True, skip_group_check=True)
                    nc.vector.tensor_add(dnb, dnb, wnps)
                    nc.vector.tensor_copy(out=dnst[:, b, :], in_=dnb[0:1, :])
                ivb = iv_sb.tile([128, H], f32, tag='ivb')
                nc.vector.reciprocal(out=ivb, in_=dnb)
                # pre-scale weights by 1/den so the w@V output is normalized
                wms = ws_p.tile([128, n_tiles, H], f32r, tag='wms')
                nc.vector.tensor_tensor(out=wms, in0=wm,
                                        in1=ivb.unsqueeze(1).broadcast_to([128, n_tiles, H]),
                                        op=ALU.mult)
                atp = at_ps.tile([H, D], f32, tag='atps')
                pe_absorb(vc, wms)
                for t in range(n_tiles):
                    nc.tensor.matmul(atp[:, :], wms[:, t, :], vc[:, t, :],
                                     start=(t == 0), stop=(t == n_tiles - 1),
                                     skip_group_check=True)
                pend.append((atp, b))
                if len(pend) > 2:
                    epilogue(*pend.pop(0))
            for p in pend:
                epilogue(*p)

        # ---------- self attention ----------
        attT_s = const.tile([128, 2, BC], f32r, tag='attT_s')
        attention(KT_S, dr['K_cache'], dr['V_cache'], qkv['q'], attT_s, False, True,
                  sk_p, sv_p, SELF_MULT_POOL)

        # new-key numerator: nv = v * w_new * inv (batch layout).
        # invb[b, h] = 1/dnst[0, b, h]: bring the denominator rows back to
        # batch-on-partition layout with 8 single-row transposes.
        pe_absorb(dnst)
        inv_ps = bc_ps.tile([BC, H], f32, tag='aux')
        for h in range(H):
            nc.tensor.transpose(inv_ps[0:BC, h:h + 1], dnst[:, :, h],
                                ident[0:1, 0:1])
        invb = const.tile([BC, H], f32, tag='invb')
        nc.vector.reciprocal(out=invb, in_=inv_ps[0:BC, :])
        nv = const.tile([BC, D], f32, tag='nv')
        nc.vector.tensor_tensor(out=nv.rearrange('p (g s) -> p g s', g=H),
                                in0=qkv['v'].rearrange('p (g s) -> p g s', g=H),
                                in1=w_new.unsqueeze(2).broadcast_to([BC, H, DH]),
                                op=ALU.mult)
        nc.vector.tensor_tensor(out=nv.rearrange('p (g s) -> p g s', g=H),
                                in0=nv.rearrange('p (g s) -> p g s', g=H),
                                in1=invb.unsqueeze(2).broadcast_to([BC, H, DH]),
                                op=ALU.mult)
        nvT = make_T(nv, 'nvT')

        # h1 = LN1(ht + att_self @ w0_s + b0_s)
        ps = linear_psum([attT_s, nvT], 'w0_s')
        h1p = const.tile([BC, D], f32, tag='h1p')
        nc.vector.tensor_add(h1p, ps, vsb['b0_s'])
        nc.vector.tensor_add(h1p, h1p, ht)
        h1 = const.tile([BC, D], f32, tag='h1')
        layernorm(h1, h1p, 'ln1_g', 'ln1_b', 'ln1')

        # ---------- cross attention ----------
        h1T = make_T(h1, 'h1T')
        psq = linear_psum([h1T], 'wq_a')
        qa = const.tile([BC, D], f32r, tag='qa')
        nc.vector.tensor_add(qa, psq, vsb['bq_a'])

        attT_a = const.tile([128, 2, BC], f32r, tag='attT_a')
        attention(KT_A, dr['K_att'], dr['V_att'], qa, attT_a, True, None,
                  ck_p, cv_p, False)

        # h2 = LN2(h1 + att_cross @ w0_a + b0_a)
        ps2 = linear_psum([attT_a], 'w0_a')
        h2p = const.tile([BC, D], f32, tag='h2p')
        nc.vector.tensor_add(h2p, ps2, vsb['b0_a'])
        nc.vector.tensor_add(h2p, h2p, h1)
        h2 = const.tile([BC, D], f32, tag='h2')
        layernorm(h2, h2p, 'ln2_g', 'ln2_b', 'ln2')

        # ---------- MLP ----------
        h2T = make_T(h2, 'h2T')
        psm = linear_psum([h2T], 'w1')
        m1 = const.tile([BC, D], f32, tag='m1')
        nc.vector.tensor_add(m1, psm, vsb['b1'])
        m1r = const.tile([BC, D], f32, tag='m1r')
        nc.scalar.activation(out=m1r, in_=m1, func=FX.Relu, scale=1.0)
        pe_absorb(m1r)
        m1T = make_T(m1r, 'm1T')
        psm2 = linear_psum([m1T], 'w2')
        h3p = const.tile([BC, D], f32, tag='h3p')
        nc.vector.tensor_add(h3p, psm2, vsb['b2'])
        nc.vector.tensor_add(h3p, h3p, h2)
        outt = const.tile([BC, D], f32, tag='outt')
        layernorm(outt, h3p, 'ln3_g', 'ln3_b', 'ln3')
        nc.sync.dma_start(out=out_dram[:, :], in_=outt)


_CACHE = {}


def _get_nc():
    if 'nc' not in _CACHE:
        _CACHE['nc'] = _build()
    return _CACHE['nc']


def _make_in_maps(inputs):
    np_in = {k: np.ascontiguousarray(np.asarray(v)) for k, v in inputs.items()}
    ident = np.eye(128, dtype=np.float32)
    in_maps = []
    for c in range(NCORES):
        sl = slice(c * BC, (c + 1) * BC)
        m = np_in['mask'][sl].astype(np.float32)          # [BC, NA], True = masked
        notm = (1.0 - m).reshape(BC, KT_A, 128).transpose(2, 1, 0).copy()  # [128, KT_A, BC]
        im = {
            'h_t': np_in['h_t'][sl],
            'K_att': np_in['K_att'][sl],
            'V_att': np_in['V_att'][sl],
            'K_cache': np_in['K_cache'][sl],
            'V_cache': np_in['V_cache'][sl],
            'notmT': notm,
            'ident': ident,
            'identr': ident,
            'onesr': np.ones((128, 1), dtype=np.float32),
        }
        for n in WNAMES + BNAMES + LNAMES:
            im[n] = np_in[n]
        in_maps.append(im)
    return in_maps


def run_on_device(inputs):
    nc = _get_nc()
    in_maps = _make_in_maps(inputs)
    res = bass_utils.run_bass_kernel_spmd(nc, in_maps, core_ids=list(range(NCORES)),
                                          trace=False)
    outs = [res.results[c]['out'] for c in range(NCORES)]
    return np.concatenate(outs, axis=0).astype(np.float32)


def kernel(**inputs):
    return run_on_device(inputs)


# revision 22
# speedup vs baseline: 100.1005x; 1.8762x over previous
import sys
if '/opt/trn_rl_repo' not in sys.path:
    sys.path.insert(0, '/opt/trn_rl_repo')
import numpy as np

import concourse.bass as bass
import concourse.bacc as bacc
import concourse.tile as tile
from concourse import mybir
from concourse import bass_utils

f32 = mybir.dt.float32
f32r = mybir.dt.float32r
bf16 = mybir.dt.bfloat16
FX = mybir.ActivationFunctionType
ALU = mybir.AluOpType
AX = mybir.AxisListType

B, D, H, DH = 256, 256, 8, 32
NCORES = 8
BC = B // NCORES          # 32 batches per core
LC = 1024                 # self-attn KV cache length
NA = 2048                 # cross-attn key count
KT_S = LC // 128          # 8 key tiles (self)
KT_A = NA // 128          # 16 key tiles (cross)
SCALE = 1.0 / float(np.sqrt(DH))
EPS = 1e-5
QLOOK = 3                 # q-broadcast prefetch distance (batches)
SELF_MULT_POOL = False    # gpsimd scalar_tensor_tensor is not compilable by walrus

WNAMES = ['wq_s', 'wk_s', 'wv_s', 'w0_s', 'wq_a', 'w0_a', 'w1', 'w2']
BNAMES = ['bq_s', 'bk_s', 'bv_s', 'b0_s', 'bq_a', 'b0_a', 'b1', 'b2']
LNAMES = ['ln1_g', 'ln1_b', 'ln2_g', 'ln2_b', 'ln3_g', 'ln3_b']


def _r(ap):
    return ap if ap.dtype == f32r else ap.bitcast(f32r)


def _build(repeat=1):
    nc = bacc.Bacc()
    dr = {}
    dr['h_t'] = nc.dram_tensor('h_t', [BC, 1, D], f32, kind='ExternalInput')
    dr['K_att'] = nc.dram_tensor('K_att', [BC, NA, D], f32, kind='ExternalInput')
    dr['V_att'] = nc.dram_tensor('V_att', [BC, NA, D], f32r, kind='ExternalInput')
    dr['K_cache'] = nc.dram_tensor('K_cache', [BC, LC, D], f32, kind='ExternalInput')
    dr['V_cache'] = nc.dram_tensor('V_cache', [BC, LC, D], f32r, kind='ExternalInput')
    dr['notmT'] = nc.dram_tensor('notmT', [128, KT_A, BC], f32, kind='ExternalInput')
    dr['ident'] = nc.dram_tensor('ident', [128, 128], f32, kind='ExternalInput')
    dr['identr'] = nc.dram_tensor('identr', [128, 128], f32r, kind='ExternalInput')
    dr['onesr'] = nc.dram_tensor('onesr', [128, 1], f32r, kind='ExternalInput')
    for n in WNAMES:
        dr[n] = nc.dram_tensor(n, [D, D], f32r, kind='ExternalInput')
    for n in BNAMES + LNAMES:
        dr[n] = nc.dram_tensor(n, [D], f32, kind='ExternalInput')
    out = nc.dram_tensor('out', [BC, D], f32, kind='ExternalOutput')

    with tile.TileContext(nc) as tc:
        for _ in range(repeat):
            _emit(nc, tc, dr, out)
    nc.compile()
    return nc


def _emit(nc, tc, dr, out_dram):
    import contextlib
    ctx = contextlib.ExitStack()
    with ctx:
        const = ctx.enter_context(tc.tile_pool(name='const', bufs=1))
        ck_p = ctx.enter_context(tc.tile_pool(name='ck', bufs=3))
        cv_p = ctx.enter_context(tc.tile_pool(name='cv', bufs=3))
        sk_p = ctx.enter_context(tc.tile_pool(name='sk', bufs=2))
        sv_p = ctx.enter_context(tc.tile_pool(name='sv', bufs=2))
        qb_p = ctx.enter_context(tc.tile_pool(name='qb', bufs=QLOOK + 2))
        pr_p = ctx.enter_context(tc.tile_pool(name='pr', bufs=2))
        sc_p = ctx.enter_context(tc.tile_pool(name='sc', bufs=2))
        w_p = ctx.enter_context(tc.tile_pool(name='w', bufs=2))
        wm_p = ctx.enter_context(tc.tile_pool(name='wm', bufs=2))
        ws_p = ctx.enter_context(tc.tile_pool(name='ws', bufs=2))
        dn_sb = ctx.enter_context(tc.tile_pool(name='dnsb', bufs=2))
        iv_sb = ctx.enter_context(tc.tile_pool(name='ivsb', bufs=2))
        at_sb = ctx.enter_context(tc.tile_pool(name='atsb', bufs=3))
        tr_ps = ctx.enter_context(tc.tile_pool(name='trps', bufs=1, space='PSUM'))
        aux_ps = ctx.enter_context(tc.tile_pool(name='auxps', bufs=1, space='PSUM'))
        at_ps = ctx.enter_context(tc.tile_pool(name='atps', bufs=3, space='PSUM'))
        dn_ps = ctx.enter_context(tc.tile_pool(name='dnps', bufs=2, space='PSUM'))
        bc_ps = aux_ps
        ln_ps = aux_ps

        garb = tr_ps.tile([1, 1], f32, tag='garb')

        def pe_absorb(*aps):
            # PE matmul/transpose can carry only ONE sem wait in its LW slot.
            # Before a matmul whose deps span several producers, emit 1x1
            # self-matmuls so the PE observes those sems here instead.
            for a in aps:
                if a is None:
                    continue
                e = a[tuple(slice(0, 1) for _ in range(len(a.shape)))]
                if e.dtype == f32r:
                    e = e.bitcast(f32)
                nc.tensor.matmul(garb[:, :], e, e, start=True, stop=True,
                                 skip_group_check=True)

        # ---------- persistent loads ----------
        ident = const.tile([128, 128], f32, tag='ident')
        nc.sync.dma_start(out=ident, in_=dr['ident'][:, :])
        pe_absorb(ident)
        identr = const.tile([128, 128], f32r, tag='identr')
        nc.sync.dma_start(out=identr, in_=dr['identr'][:, :])
        onesr = const.tile([128, 1], f32r, tag='onesr')
        nc.sync.dma_start(out=onesr, in_=dr['onesr'][:, :])
        epst = const.tile([BC, 1], f32, tag='epst')
        nc.vector.memset(epst, EPS)

        wsb = {}
        for n in WNAMES:
            wsb[n] = const.tile([128, 2, D], f32r, tag='w_' + n, name='w_' + n)
            nc.sync.dma_start(out=wsb[n], in_=dr[n][:, :].rearrange('(t p) j -> p t j', p=128))
        vsb = {}
        for n in BNAMES + LNAMES:
            vsb[n] = const.tile([BC, D], f32, tag='v_' + n, name='v_' + n)
            nc.gpsimd.dma_start(out=vsb[n], in_=dr[n][:].unsqueeze(0).to_broadcast([BC, D]))

        notmT = const.tile([128, KT_A, BC], f32, tag='notmT')
        nc.sync.dma_start(out=notmT, in_=dr['notmT'][:, :, :])

        ht = const.tile([BC, D], f32, tag='ht')
        nc.sync.dma_start(out=ht, in_=dr['h_t'][:, 0, :])
        pe_absorb(ht)

        # ---------- helpers ----------
        def transpose_128(dst, src, cols):
            rows = src.shape[0]
            ps = tr_ps.tile([128, 128], f32, tag='trps')
            nc.tensor.transpose(ps[0:cols, 0:rows], src, ident[0:rows, 0:rows])
            nc.vector.tensor_copy(out=dst, in_=ps[0:cols, 0:rows])

        def make_T(src_f32, tagname):
            dstT = const.tile([128, 2, BC], f32r, tag=tagname, name=tagname)
            for t in range(2):
                transpose_128(dstT[:, t, :], src_f32[:, 128 * t:128 * (t + 1)], 128)
            return dstT

        def linear_psum(srcT_list, wname):
            ps = ln_ps.tile([BC, D], f32, tag='aux')
            pe_absorb(wsb[wname])
            n_mm = 2 * len(srcT_list)
            i = 0
            for srcT in srcT_list:
                for t in range(2):
                    nc.tensor.matmul(ps[:, :], _r(srcT[:, t, :]), wsb[wname][:, t, :],
                                     start=(i == 0), stop=(i == n_mm - 1))
                    i += 1
            return ps

        def layernorm(dst, src, gname, bname, tagp):
            stats = const.tile([BC, 6], f32, tag=tagp + '_st', name=tagp + '_st')
            nc.vector.bn_stats(out=stats, in_=src)
            mv = const.tile([BC, 2], f32, tag=tagp + '_mv', name=tagp + '_mv')
            nc.vector.bn_aggr(out=mv, in_=stats)
            sd = const.tile([BC, 1], f32, tag=tagp + '_sd', name=tagp + '_sd')
            nc.scalar.activation(out=sd, in_=mv[:, 1:2], func=FX.Sqrt,
                                 bias=epst[:, :], scale=1.0)
            rstd = const.tile([BC, 1], f32, tag=tagp + '_rs', name=tagp + '_rs')
            nc.vector.reciprocal(out=rstd, in_=sd)
            nc.vector.tensor_scalar(out=dst, in0=src, scalar1=mv[:, 0:1], scalar2=rstd,
                                    op0=ALU.subtract, op1=ALU.mult)
            nc.vector.tensor_mul(dst, dst, vsb[gname])
            nc.vector.tensor_add(dst, dst, vsb[bname])

        def build_qb(q_sb, b):
            # qb[b] = q_sb[b, :] replicated across all 128 partitions, via a
            # selector matmul: lhsT = e_b (col b of identity) broadcast along
            # the free axis, so out[p, d] = sum_c delta(c, b) q[c, d] = q[b, d].
            ps = bc_ps.tile([128, D], f32, tag='aux')
            sel = identr[0:BC, b:b + 1].broadcast_to([BC, 128])
            nc.tensor.matmul(ps[:, :], sel, q_sb, start=True, stop=True,
                             skip_group_check=True)
            qb = qb_p.tile([128, D], f32, tag='qb')
            nc.scalar.copy(out=qb, in_=ps[:, :])
            return qb

        # ---------- qkv for self-attn ----------
        htT = make_T(ht, 'htT')
        qkv = {}
        for nm, wn, bn in (('q', 'wq_s', 'bq_s'), ('k', 'wk_s', 'bk_s'), ('v', 'wv_s', 'bv_s')):
            ps = linear_psum([htT], wn)
            dt = f32r if nm == 'q' else f32
            qkv[nm] = const.tile([BC, D], dt, tag='qkv_' + nm, name='qkv_' + nm)
            nc.vector.tensor_add(qkv[nm], ps, vsb[bn])

        # new-key (appended k/v) terms, all-batch
        qk = const.tile([BC, D], f32, tag='qk')
        nc.vector.tensor_mul(qk, qkv['q'], qkv['k'])
        s_new = const.tile([BC, H], f32, tag='s_new')
        nc.vector.reduce_sum(out=s_new, in_=qk.rearrange('p (g s) -> p g s', g=H), axis=AX.X)
        w_new = const.tile([BC, H], f32r, tag='w_new')
        nc.scalar.activation(out=w_new, in_=s_new, func=FX.Exp, scale=SCALE)
        pe_absorb(w_new)

        # full self-attn denominators (incl. w_new), one row per batch on
        # partition 0, collected during the self loop for the nv term
        dnst = const.tile([1, BC, H], f32, tag='dnst')

        # ---------- attention inner loop ----------
        # The softmax denominator is computed with an all-ones matmul over the
        # weights (partition sums replicated on every partition), and 1/den is
        # multiplied into the weights BEFORE the w@V matmul, so its PSUM output
        # is already normalized. Nothing in the steady-state loop reads the
        # attention PSUM except one ACT copy deferred by two batches; all other
        # waits are short-range, so no engine wait-queue ever parks long.
        def attention(n_tiles, K_dram, V_dram, q_sb, attT_dst, masked, self_newkey,
                      kp, vp, pool_mult):
            pe_absorb(q_sb)
            qbs = {b: build_qb(q_sb, b) for b in range(min(QLOOK, BC))}

            def epilogue(atp, b):
                attm = at_sb.tile([H, D], f32, tag='attm')
                nc.scalar.copy(out=attm, in_=atp[:, :])
                for t in range(2):
                    pa = tr_ps.tile([128, 128], f32, tag='trps')
                    nc.tensor.transpose(pa[0:128, 0:H], attm[:, 128 * t:128 * (t + 1)],
                                        ident[0:H, 0:H])
                    for k in range(4):
                        h = 4 * t + k
                        nc.scalar.copy(out=attT_dst[32 * k:32 * (k + 1), t, b:b + 1],
                                       in_=pa[32 * k:32 * (k + 1), h:h + 1])

            pend = []
            for b in range(BC):
                if b + QLOOK < BC:
                    qbs[b + QLOOK] = build_qb(q_sb, b + QLOOK)
                kc = kp.tile([128, n_tiles, D], f32, tag='kc')
                nc.sync.dma_start(out=kc,
                                  in_=K_dram[b].rearrange('(t p) d -> p t d', p=128))
                vc = vp.tile([128, n_tiles, D], f32r, tag='vc')
                nc.sync.dma_start(out=vc,
                                  in_=V_dram[b].rearrange('(t p) d -> p t d', p=128))
                qb = qbs.pop(b)
                # scores^T[k, t, h] = sum_s K[k, t, h*32+s] * q[h*32+s] * SCALE
                prod = pr_p.tile([128, n_tiles, D], bf16, tag='prod')
                qb_b = qb.unsqueeze(1).broadcast_to([128, n_tiles, D])
                if pool_mult:
                    nc.gpsimd.scalar_tensor_tensor(out=prod, in0=kc, scalar=SCALE,
                                                   in1=qb_b, op0=ALU.mult, op1=ALU.mult)
                else:
                    nc.vector.scalar_tensor_tensor(out=prod, in0=kc, scalar=SCALE,
                                                   in1=qb_b, op0=ALU.mult, op1=ALU.mult)
                sc = sc_p.tile([128, n_tiles, H], f32, tag='sc')
                nc.vector.tensor_reduce(out=sc,
                                        in_=prod.rearrange('p t (h s) -> p t h s', h=H),
                                        axis=AX.X, op=ALU.add)
                w = w_p.tile([128, n_tiles, H], f32r, tag='w')
                nc.scalar.activation(out=w, in_=sc, func=FX.Exp, scale=1.0)
                if masked:
                    wm = wm_p.tile([128, n_tiles, H], f32r, tag='wm')
                    nc.vector.tensor_tensor(
                        out=wm, in0=w,
                        in1=notmT[:, 0:n_tiles, b:b + 1].broadcast_to([128, n_tiles, H]),
                        op=ALU.mult)
                else:
                    wm = w
                # denominator: all-ones matmul -> per-(t,h) partition sums on
                # every partition, laid out h-major so the t-reduce is innermost
                dps = dn_ps.tile([128, H, n_tiles], f32, tag='dn')
                nc.tensor.matmul(dps[:, :, :],
                                 onesr.broadcast_to([128, 128]),
                                 wm.rearrange('p t h -> p h t'),
                                 start=True, stop=True, skip_group_check=True)
                dnb = dn_sb.tile([128, H], f32, tag='dnb')
                nc.vector.tensor_reduce(out=dnb, in_=dps, axis=AX.X, op=ALU.add)
                if self_newkey:
                    wnps = bc_ps.tile([128, H], f32, tag='aux')
                    seln = identr[0:BC, b:b + 1].broadcast_to([BC, 128])
                    nc.tensor.matmul(wnps[:, :], seln, w_new, start=True,
                                     stop=True, skip_group_check=True)
                    nc.vector.tensor_add(dnb, dnb, wnps)
                    nc.vector.tensor_copy(out=dnst[:, b, :], in_=dnb[0:1, :])
                ivb = iv_sb.tile([128, H], f32, tag='ivb')
                nc.vector.reciprocal(out=ivb, in_=dnb)
                # pre-scale weights by 1/den so the w@V output is normalized
                wms = ws_p.tile([128, n_tiles, H], f32r, tag='wms')
                nc.vector.tensor_tensor(out=wms, in0=wm,
                                        in1=ivb.unsqueeze(1).broadcast_to([128, n_tiles, H]),
                                        op=ALU.mult)
                atp = at_ps.tile([H, D], f32, tag='atps')
                pe_absorb(vc, wms)
                for t in range(n_tiles):
                    nc.tensor.matmul(atp[:, :], wms[:, t, :], vc[:, t, :],
                                     start=(t == 0), stop=(t == n_tiles - 1),
                                     skip_group_check=True)
                pend.append((atp, b))
                if len(pend) > 2:
                    epilogue(*pend.pop(0))
            for p in pend:
                epilogue(*p)

        # ---------- self attention ----------
        attT_s = const.tile([128, 2, BC], f32r, tag='attT_s')
        attention(KT_S, dr['K_cache'], dr['V_cache'], qkv['q'], attT_s, False, True,
                  sk_p, sv_p, SELF_MULT_POOL)

        # new-key numerator: nv = v * w_new * inv (batch layout).
        # invb[b, h] = 1/dnst[0, b, h]: bring the denominator rows back to
        # batch-on-partition layout with 8 single-row transposes.
        pe_absorb(dnst)
        inv_ps = bc_ps.tile([BC, H], f32, tag='aux')
        for h in range(H):
            nc.tensor.transpose(inv_ps[0:BC, h:h + 1], dnst[:, :, h],
                                ident[0:1, 0:1])
        invb = const.tile([BC, H], f32, tag='invb')
        nc.vector.reciprocal(out=invb, in_=inv_ps[0:BC, :])
        nv = const.tile([BC, D], f32, tag='nv')
        nc.vector.tensor_tensor(out=nv.rearrange('p (g s) -> p g s', g=H),
                                in0=qkv['v'].rearrange('p (g s) -> p g s', g=H),
                                in1=w_new.unsqueeze(2).broadcast_to([BC, H, DH]),
                                op=ALU.mult)
        nc.vector.tensor_tensor(out=nv.rearrange('p (g s) -> p g s', g=H),
                                in0=nv.rearrange('p (g s) -> p g s', g=H),
                                in1=invb.unsqueeze(2).broadcast_to([BC, H, DH]),
                                op=ALU.mult)
        nvT = make_T(nv, 'nvT')

        # h1 = LN1(ht + att_self @ w0_s + b0_s)
        ps = linear_psum([attT_s, nvT], 'w0_s')
        h1p = const.tile([BC, D], f32, tag='h1p')
        nc.vector.tensor_add(h1p, ps, vsb['b0_s'])
        nc.vector.tensor_add(h1p, h1p, ht)
        h1 = const.tile([BC, D], f32, tag='h1')
        layernorm(h1, h1p, 'ln1_g', 'ln1_b', 'ln1')

        # ---------- cross attention ----------
        h1T = make_T(h1, 'h1T')
        psq = linear_psum([h1T], 'wq_a')
        qa = const.tile([BC, D], f32r, tag='qa')
        nc.vector.tensor_add(qa, psq, vsb['bq_a'])

        attT_a = const.tile([128, 2, BC], f32r, tag='attT_a')
        attention(KT_A, dr['K_att'], dr['V_att'], qa, attT_a, True, None,
                  ck_p, cv_p, False)

        # h2 = LN2(h1 + att_cross @ w0_a + b0_a)
        ps2 = linear_psum([attT_a], 'w0_a')
        h2p = const.tile([BC, D], f32, tag='h2p')
        nc.vector.tensor_add(h2p, ps2, vsb['b0_a'])
        nc.vector.tensor_add(h2p, h2p, h1)
        h2 = const.tile([BC, D], f32, tag='h2')
        layernorm(h2, h2p, 'ln2_g', 'ln2_b', 'ln2')

        # ---------- MLP ----------
        h2T = make_T(h2, 'h2T')
        psm = linear_psum([h2T], 'w1')
        m1 = const.tile([BC, D], f32, tag='m1')
        nc.vector.tensor_add(m1, psm, vsb['b1'])
        m1r = const.tile([BC, D], f32, tag='m1r')
        nc.scalar.activation(out=m1r, in_=m1, func=FX.Relu, scale=1.0)
        pe_absorb(m1r)
        m1T = make_T(m1r, 'm1T')
        psm2 = linear_psum([m1T], 'w2')
        h3p = const.tile([BC, D], f32, tag='h3p')
        nc.vector.tensor_add(h3p, psm2, vsb['b2'])
        nc.vector.tensor_add(h3p, h3p, h2)
        outt = const.tile([BC, D], f32, tag='outt')
        layernorm(outt, h3p, 'ln3_g', 'ln3_b', 'ln3')
        nc.sync.dma_start(out=out_dram[:, :], in_=outt)


_CACHE = {}


def _get_nc():
    if 'nc' not in _CACHE:
        _CACHE['nc'] = _build()
    return _CACHE['nc']


def _make_in_maps(inputs):
    np_in = {k: np.ascontiguousarray(np.asarray(v)) for k, v in inputs.items()}
    ident = np.eye(128, dtype=np.float32)
    in_maps = []
    for c in range(NCORES):
        sl = slice(c * BC, (c + 1) * BC)
        m = np_in['mask'][sl].astype(np.float32)          # [BC, NA], True = masked
        notm = (1.0 - m).reshape(BC, KT_A, 128).transpose(2, 1, 0).copy()  # [128, KT_A, BC]
        im = {
            'h_t': np_in['h_t'][sl],
            'K_att': np_in['K_att'][sl],
            'V_att': np_in['V_att'][sl],
            'K_cache': np_in['K_cache'][sl],
            'V_cache': np_in['V_cache'][sl],
            'notmT': notm,
            'ident': ident,
            'identr': ident,
            'onesr': np.ones((128, 1), dtype=np.float32),
        }
        for n in WNAMES + BNAMES + LNAMES:
            im[n] = np_in[n]
        in_maps.append(im)
    return in_maps


def run_on_device(inputs):
    nc = _get_nc()
    in_maps = _make_in_maps(inputs)
    res = bass_utils.run_bass_kernel_spmd(nc, in_maps, core_ids=list(range(NCORES)),
                                          trace=False)
    outs = [res.results[c]['out'] for c in range(NCORES)]
    return np.concatenate(outs, axis=0).astype(np.float32)


def kernel(**inputs):
    return run_on_device(inputs)


# revision 23
# speedup vs baseline: 105.5415x; 1.0544x over previous
import sys
if '/opt/trn_rl_repo' not in sys.path:
    sys.path.insert(0, '/opt/trn_rl_repo')
import numpy as np

import concourse.bass as bass
import concourse.bacc as bacc
import concourse.tile as tile
from concourse import mybir
from concourse import bass_utils

f32 = mybir.dt.float32
f32r = mybir.dt.float32r
bf16 = mybir.dt.bfloat16
FX = mybir.ActivationFunctionType
ALU = mybir.AluOpType
AX = mybir.AxisListType

B, D, H, DH = 256, 256, 8, 32
NCORES = 8
BC = B // NCORES          # 32 batches per core
LC = 1024                 # self-attn KV cache length
NA = 2048                 # cross-attn key count
KT_S = LC // 128          # 8 key tiles (self)
KT_A = NA // 128          # 16 key tiles (cross)
SCALE = 1.0 / float(np.sqrt(DH))
EPS = 1e-5
QLOOK = 3                 # q-broadcast prefetch distance (batches)
SELF_MULT_POOL = False    # gpsimd scalar_tensor_tensor is not compilable by walrus

WNAMES = ['wq_s', 'wk_s', 'wv_s', 'w0_s', 'wq_a', 'w0_a', 'w1', 'w2']
BNAMES = ['bq_s', 'bk_s', 'bv_s', 'b0_s', 'bq_a', 'b0_a', 'b1', 'b2']
LNAMES = ['ln1_g', 'ln1_b', 'ln2_g', 'ln2_b', 'ln3_g', 'ln3_b']


def _r(ap):
    return ap if ap.dtype == f32r else ap.bitcast(f32r)


def _build(repeat=1):
    nc = bacc.Bacc()
    dr = {}
    dr['h_t'] = nc.dram_tensor('h_t', [BC, 1, D], f32, kind='ExternalInput')
    dr['K_att'] = nc.dram_tensor('K_att', [BC, NA, D], f32, kind='ExternalInput')
    dr['V_att'] = nc.dram_tensor('V_att', [BC, NA, D], f32r, kind='ExternalInput')
    dr['K_cache'] = nc.dram_tensor('K_cache', [BC, LC, D], f32, kind='ExternalInput')
    dr['V_cache'] = nc.dram_tensor('V_cache', [BC, LC, D], f32r, kind='ExternalInput')
    dr['notmT'] = nc.dram_tensor('notmT', [128, KT_A, BC], f32, kind='ExternalInput')
    dr['ident'] = nc.dram_tensor('ident', [128, 128], f32, kind='ExternalInput')
    dr['identr'] = nc.dram_tensor('identr', [128, 128], f32r, kind='ExternalInput')
    dr['onesr'] = nc.dram_tensor('onesr', [128, 1], f32r, kind='ExternalInput')
    for n in WNAMES:
        dr[n] = nc.dram_tensor(n, [D, D], f32r, kind='ExternalInput')
    for n in BNAMES + LNAMES:
        dr[n] = nc.dram_tensor(n, [D], f32, kind='ExternalInput')
    out = nc.dram_tensor('out', [BC, D], f32, kind='ExternalOutput')

    with tile.TileContext(nc) as tc:
        for _ in range(repeat):
            _emit(nc, tc, dr, out)
    nc.compile()
    return nc


def _emit(nc, tc, dr, out_dram):
    import contextlib
    ctx = contextlib.ExitStack()
    with ctx:
        const = ctx.enter_context(tc.tile_pool(name='const', bufs=1))
        ck_p = ctx.enter_context(tc.tile_pool(name='ck', bufs=3))
        cv_p = ctx.enter_context(tc.tile_pool(name='cv', bufs=3))
        sk_p = ctx.enter_context(tc.tile_pool(name='sk', bufs=2))
        sv_p = ctx.enter_context(tc.tile_pool(name='sv', bufs=2))
        qb_p = ctx.enter_context(tc.tile_pool(name='qb', bufs=QLOOK + 2))
        pr_p = ctx.enter_context(tc.tile_pool(name='pr', bufs=2))
        sc_p = ctx.enter_context(tc.tile_pool(name='sc', bufs=2))
        w_p = ctx.enter_context(tc.tile_pool(name='w', bufs=2))
        wm_p = ctx.enter_context(tc.tile_pool(name='wm', bufs=2))
        ws_p = ctx.enter_context(tc.tile_pool(name='ws', bufs=2))
        dn_sb = ctx.enter_context(tc.tile_pool(name='dnsb', bufs=2))
        iv_sb = ctx.enter_context(tc.tile_pool(name='ivsb', bufs=2))
        at_sb = ctx.enter_context(tc.tile_pool(name='atsb', bufs=3))
        tr_ps = ctx.enter_context(tc.tile_pool(name='trps', bufs=1, space='PSUM'))
        aux_ps = ctx.enter_context(tc.tile_pool(name='auxps', bufs=1, space='PSUM'))
        at_ps = ctx.enter_context(tc.tile_pool(name='atps', bufs=3, space='PSUM'))
        dn_ps = ctx.enter_context(tc.tile_pool(name='dnps', bufs=2, space='PSUM'))
        bc_ps = aux_ps
        ln_ps = aux_ps

        garb = tr_ps.tile([1, 1], f32, tag='garb')

        def pe_absorb(*aps):
            # PE matmul/transpose can carry only ONE sem wait in its LW slot.
            # Before a matmul whose deps span several producers, emit 1x1
            # self-matmuls so the PE observes those sems here instead.
            for a in aps:
                if a is None:
                    continue
                e = a[tuple(slice(0, 1) for _ in range(len(a.shape)))]
                if e.dtype == f32r:
                    e = e.bitcast(f32)
                nc.tensor.matmul(garb[:, :], e, e, start=True, stop=True,
                                 skip_group_check=True)

        # ---------- persistent loads ----------
        ident = const.tile([128, 128], f32, tag='ident')
        nc.sync.dma_start(out=ident, in_=dr['ident'][:, :])
        pe_absorb(ident)
        identr = const.tile([128, 128], f32r, tag='identr')
        nc.sync.dma_start(out=identr, in_=dr['identr'][:, :])
        onesr = const.tile([128, 1], f32r, tag='onesr')
        nc.sync.dma_start(out=onesr, in_=dr['onesr'][:, :])
        epst = const.tile([BC, 1], f32, tag='epst')
        nc.vector.memset(epst, EPS)

        wsb = {}
        for n in WNAMES:
            wsb[n] = const.tile([128, 2, D], f32r, tag='w_' + n, name='w_' + n)
            nc.sync.dma_start(out=wsb[n], in_=dr[n][:, :].rearrange('(t p) j -> p t j', p=128))
        vsb = {}
        for n in BNAMES + LNAMES:
            vsb[n] = const.tile([BC, D], f32, tag='v_' + n, name='v_' + n)
            nc.gpsimd.dma_start(out=vsb[n], in_=dr[n][:].unsqueeze(0).to_broadcast([BC, D]))

        notmT = const.tile([128, KT_A, BC], f32, tag='notmT')
        nc.sync.dma_start(out=notmT, in_=dr['notmT'][:, :, :])

        ht = const.tile([BC, D], f32, tag='ht')
        nc.sync.dma_start(out=ht, in_=dr['h_t'][:, 0, :])
        pe_absorb(ht)

        # ---------- helpers ----------
        def transpose_128(dst, src, cols):
            rows = src.shape[0]
            ps = tr_ps.tile([128, 128], f32, tag='trps')
            nc.tensor.transpose(ps[0:cols, 0:rows], src, ident[0:rows, 0:rows])
            nc.vector.tensor_copy(out=dst, in_=ps[0:cols, 0:rows])

        def make_T(src_f32, tagname):
            dstT = const.tile([128, 2, BC], f32r, tag=tagname, name=tagname)
            for t in range(2):
                transpose_128(dstT[:, t, :], src_f32[:, 128 * t:128 * (t + 1)], 128)
            return dstT

        def linear_psum(srcT_list, wname):
            ps = ln_ps.tile([BC, D], f32, tag='aux')
            pe_absorb(wsb[wname])
            n_mm = 2 * len(srcT_list)
            i = 0
            for srcT in srcT_list:
                for t in range(2):
                    nc.tensor.matmul(ps[:, :], _r(srcT[:, t, :]), wsb[wname][:, t, :],
                                     start=(i == 0), stop=(i == n_mm - 1))
                    i += 1
            return ps

        def layernorm(dst, src, gname, bname, tagp):
            stats = const.tile([BC, 6], f32, tag=tagp + '_st', name=tagp + '_st')
            nc.vector.bn_stats(out=stats, in_=src)
            mv = const.tile([BC, 2], f32, tag=tagp + '_mv', name=tagp + '_mv')
            nc.vector.bn_aggr(out=mv, in_=stats)
            sd = const.tile([BC, 1], f32, tag=tagp + '_sd', name=tagp + '_sd')
            nc.scalar.activation(out=sd, in_=mv[:, 1:2], func=FX.Sqrt,
                                 bias=epst[:, :], scale=1.0)
            rstd = const.tile([BC, 1], f32, tag=tagp + '_rs', name=tagp + '_rs')
            nc.vector.reciprocal(out=rstd, in_=sd)
            nc.vector.tensor_scalar(out=dst, in0=src, scalar1=mv[:, 0:1], scalar2=rstd,
                                    op0=ALU.subtract, op1=ALU.mult)
            nc.vector.tensor_mul(dst, dst, vsb[gname])
            nc.vector.tensor_add(dst, dst, vsb[bname])

        def build_qb(q_sb, b):
            # qb[b] = q_sb[b, :] replicated across all 128 partitions, via a
            # selector matmul: lhsT = e_b (col b of identity) broadcast along
            # the free axis, so out[p, d] = sum_c delta(c, b) q[c, d] = q[b, d].
            ps = bc_ps.tile([128, D], f32, tag='aux')
            sel = identr[0:BC, b:b + 1].broadcast_to([BC, 128])
            nc.tensor.matmul(ps[:, :], sel, q_sb, start=True, stop=True,
                             skip_group_check=True)
            qb = qb_p.tile([128, D], f32, tag='qb')
            nc.scalar.copy(out=qb, in_=ps[:, :])
            return qb

        # ---------- qkv for self-attn ----------
        htT = make_T(ht, 'htT')
        qkv = {}
        for nm, wn, bn in (('q', 'wq_s', 'bq_s'), ('k', 'wk_s', 'bk_s'), ('v', 'wv_s', 'bv_s')):
            ps = linear_psum([htT], wn)
            dt = f32r if nm == 'q' else f32
            qkv[nm] = const.tile([BC, D], dt, tag='qkv_' + nm, name='qkv_' + nm)
            nc.vector.tensor_add(qkv[nm], ps, vsb[bn])

        # new-key (appended k/v) terms, all-batch
        qk = const.tile([BC, D], f32, tag='qk')
        nc.vector.tensor_mul(qk, qkv['q'], qkv['k'])
        s_new = const.tile([BC, H], f32, tag='s_new')
        nc.vector.reduce_sum(out=s_new, in_=qk.rearrange('p (g s) -> p g s', g=H), axis=AX.X)
        w_new = const.tile([BC, H], f32r, tag='w_new')
        nc.scalar.activation(out=w_new, in_=s_new, func=FX.Exp, scale=SCALE)
        pe_absorb(w_new)

        # full self-attn denominators (incl. w_new), one row per batch on
        # partition 0, collected during the self loop for the nv term
        dnst = const.tile([1, BC, H], f32, tag='dnst')

        # ---------- attention inner loop ----------
        # The softmax denominator is computed with an all-ones matmul over the
        # weights (partition sums replicated on every partition), and 1/den is
        # multiplied into the weights BEFORE the w@V matmul, so its PSUM output
        # is already normalized. Nothing in the steady-state loop reads the
        # attention PSUM except one ACT copy deferred by two batches; all other
        # waits are short-range, so no engine wait-queue ever parks long.
        def attention(n_tiles, K_dram, V_dram, q_sb, attT_dst, masked, self_newkey,
                      kp, vp, pool_mult):
            pe_absorb(q_sb)
            qbs = {b: build_qb(q_sb, b) for b in range(min(QLOOK, BC))}

            def epilogue(atp, b):
                attm = at_sb.tile([H, D], f32, tag='attm')
                nc.scalar.copy(out=attm, in_=atp[:, :])
                for t in range(2):
                    pa = tr_ps.tile([128, 128], f32, tag='trps')
                    nc.tensor.transpose(pa[0:128, 0:H], attm[:, 128 * t:128 * (t + 1)],
                                        ident[0:H, 0:H])
                    for k in range(4):
                        h = 4 * t + k
                        nc.scalar.copy(out=attT_dst[32 * k:32 * (k + 1), t, b:b + 1],
                                       in_=pa[32 * k:32 * (k + 1), h:h + 1])

            pend = []
            for b in range(BC):
                if b + QLOOK < BC:
                    qbs[b + QLOOK] = build_qb(q_sb, b + QLOOK)
                kc = kp.tile([128, n_tiles, D], f32, tag='kc')
                nc.sync.dma_start(out=kc,
                                  in_=K_dram[b].rearrange('(p t) d -> p t d', p=128))
                vc = vp.tile([128, n_tiles, D], f32r, tag='vc')
                nc.sync.dma_start(out=vc,
                                  in_=V_dram[b].rearrange('(p t) d -> p t d', p=128))
                qb = qbs.pop(b)
                # scores^T[k, t, h] = sum_s K[k, t, h*32+s] * q[h*32+s] * SCALE
                prod = pr_p.tile([128, n_tiles, D], bf16, tag='prod')
                qb_b = qb.unsqueeze(1).broadcast_to([128, n_tiles, D])
                if pool_mult:
                    nc.gpsimd.scalar_tensor_tensor(out=prod, in0=kc, scalar=SCALE,
                                                   in1=qb_b, op0=ALU.mult, op1=ALU.mult)
                else:
                    nc.vector.scalar_tensor_tensor(out=prod, in0=kc, scalar=SCALE,
                                                   in1=qb_b, op0=ALU.mult, op1=ALU.mult)
                sc = sc_p.tile([128, n_tiles, H], f32, tag='sc')
                nc.vector.tensor_reduce(out=sc,
                                        in_=prod.rearrange('p t (h s) -> p t h s', h=H),
                                        axis=AX.X, op=ALU.add)
                w = w_p.tile([128, n_tiles, H], f32r, tag='w')
                nc.scalar.activation(out=w, in_=sc, func=FX.Exp, scale=1.0)
                if masked:
                    wm = wm_p.tile([128, n_tiles, H], f32r, tag='wm')
                    nc.vector.tensor_tensor(
                        out=wm, in0=w,
                        in1=notmT[:, 0:n_tiles, b:b + 1].broadcast_to([128, n_tiles, H]),
                        op=ALU.mult)
                else:
                    wm = w
                # denominator: all-ones matmul -> per-(t,h) partition sums on
                # every partition, laid out h-major so the t-reduce is innermost
                dps = dn_ps.tile([128, H, n_tiles], f32, tag='dn')
                nc.tensor.matmul(dps[:, :, :],
                                 onesr.broadcast_to([128, 128]),
                                 wm.rearrange('p t h -> p h t'),
                                 start=True, stop=True, skip_group_check=True)
                dnb = dn_sb.tile([128, H], f32, tag='dnb')
                nc.vector.tensor_reduce(out=dnb, in_=dps, axis=AX.X, op=ALU.add)
                if self_newkey:
                    wnps = bc_ps.tile([128, H], f32, tag='aux')
                    seln = identr[0:BC, b:b + 1].broadcast_to([BC, 128])
                    nc.tensor.matmul(wnps[:, :], seln, w_new, start=True,
                                     stop=True, skip_group_check=True)
                    nc.vector.tensor_add(dnb, dnb, wnps)
                    nc.vector.tensor_copy(out=dnst[:, b, :], in_=dnb[0:1, :])
                ivb = iv_sb.tile([128, H], f32, tag='ivb')
                nc.vector.reciprocal(out=ivb, in_=dnb)
                # pre-scale weights by 1/den so the w@V output is normalized
                wms = ws_p.tile([128, n_tiles, H], f32r, tag='wms')
                nc.vector.tensor_tensor(out=wms, in0=wm,
                                        in1=ivb.unsqueeze(1).broadcast_to([128, n_tiles, H]),
                                        op=ALU.mult)
                atp = at_ps.tile([H, D], f32, tag='atps')
                pe_absorb(vc, wms)
                for t in range(n_tiles):
                    nc.tensor.matmul(atp[:, :], wms[:, t, :], vc[:, t, :],
                                     start=(t == 0), stop=(t == n_tiles - 1),
                                     skip_group_check=True)
                pend.append((atp, b))
                if len(pend) > 2:
                    epilogue(*pend.pop(0))
            for p in pend:
                epilogue(*p)

        # ---------- self attention ----------
        attT_s = const.tile([128, 2, BC], f32r, tag='attT_s')
        attention(KT_S, dr['K_cache'], dr['V_cache'], qkv['q'], attT_s, False, True,
                  sk_p, sv_p, SELF_MULT_POOL)

        # new-key numerator: nv = v * w_new * inv (batch layout).
        # invb[b, h] = 1/dnst[0, b, h]: bring the denominator rows back to
        # batch-on-partition layout with 8 single-row transposes.
        pe_absorb(dnst)
        inv_ps = bc_ps.tile([BC, H], f32, tag='aux')
        for h in range(H):
            nc.tensor.transpose(inv_ps[0:BC, h:h + 1], dnst[:, :, h],
                                ident[0:1, 0:1])
        invb = const.tile([BC, H], f32, tag='invb')
        nc.vector.reciprocal(out=invb, in_=inv_ps[0:BC, :])
        nv = const.tile([BC, D], f32, tag='nv')
        nc.vector.tensor_tensor(out=nv.rearrange('p (g s) -> p g s', g=H),
                                in0=qkv['v'].rearrange('p (g s) -> p g s', g=H),
                                in1=w_new.unsqueeze(2).broadcast_to([BC, H, DH]),
                                op=ALU.mult)
        nc.vector.tensor_tensor(out=nv.rearrange('p (g s) -> p g s', g=H),
                                in0=nv.rearrange('p (g s) -> p g s', g=H),
                                in1=invb.unsqueeze(2).broadcast_to([BC, H, DH]),
                                op=ALU.mult)
        nvT = make_T(nv, 'nvT')

        # h1 = LN1(ht + att_self @ w0_s + b0_s)
        ps = linear_psum([attT_s, nvT], 'w0_s')
        h1p = const.tile([BC, D], f32, tag='h1p')
        nc.vector.tensor_add(h1p, ps, vsb['b0_s'])
        nc.vector.tensor_add(h1p, h1p, ht)
        h1 = const.tile([BC, D], f32, tag='h1')
        layernorm(h1, h1p, 'ln1_g', 'ln1_b', 'ln1')

        # ---------- cross attention ----------
        h1T = make_T(h1, 'h1T')
        psq = linear_psum([h1T], 'wq_a')
        qa = const.tile([BC, D], f32r, tag='qa')
        nc.vector.tensor_add(qa, psq, vsb['bq_a'])

        attT_a = const.tile([128, 2, BC], f32r, tag='attT_a')
        attention(KT_A, dr['K_att'], dr['V_att'], qa, attT_a, True, None,
                  ck_p, cv_p, False)

        # h2 = LN2(h1 + att_cross @ w0_a + b0_a)
        ps2 = linear_psum([attT_a], 'w0_a')
        h2p = const.tile([BC, D], f32, tag='h2p')
        nc.vector.tensor_add(h2p, ps2, vsb['b0_a'])
        nc.vector.tensor_add(h2p, h2p, h1)
        h2 = const.tile([BC, D], f32, tag='h2')
        layernorm(h2, h2p, 'ln2_g', 'ln2_b', 'ln2')

        # ---------- MLP ----------
        h2T = make_T(h2, 'h2T')
        psm = linear_psum([h2T], 'w1')
        m1 = const.tile([BC, D], f32, tag='m1')
        nc.vector.tensor_add(m1, psm, vsb['b1'])
        m1r = const.tile([BC, D], f32, tag='m1r')
        nc.scalar.activation(out=m1r, in_=m1, func=FX.Relu, scale=1.0)
        pe_absorb(m1r)
        m1T = make_T(m1r, 'm1T')
        psm2 = linear_psum([m1T], 'w2')
        h3p = const.tile([BC, D], f32, tag='h3p')
        nc.vector.tensor_add(h3p, psm2, vsb['b2'])
        nc.vector.tensor_add(h3p, h3p, h2)
        outt = const.tile([BC, D], f32, tag='outt')
        layernorm(outt, h3p, 'ln3_g', 'ln3_b', 'ln3')
        nc.sync.dma_start(out=out_dram[:, :], in_=outt)


_CACHE = {}


def _get_nc():
    if 'nc' not in _CACHE:
        _CACHE['nc'] = _build()
    return _CACHE['nc']


def _make_in_maps(inputs):
    np_in = {k: np.ascontiguousarray(np.asarray(v)) for k, v in inputs.items()}
    ident = np.eye(128, dtype=np.float32)
    in_maps = []
    for c in range(NCORES):
        sl = slice(c * BC, (c + 1) * BC)
        m = np_in['mask'][sl].astype(np.float32)          # [BC, NA], True = masked
        # key index = p * KT_A + t (contiguous per-partition key blocks)
        notm = (1.0 - m).reshape(BC, 128, KT_A).transpose(1, 2, 0).copy()  # [128, KT_A, BC]
        im = {
            'h_t': np_in['h_t'][sl],
            'K_att': np_in['K_att'][sl],
            'V_att': np_in['V_att'][sl],
            'K_cache': np_in['K_cache'][sl],
            'V_cache': np_in['V_cache'][sl],
            'notmT': notm,
            'ident': ident,
            'identr': ident,
            'onesr': np.ones((128, 1), dtype=np.float32),
        }
        for n in WNAMES + BNAMES + LNAMES:
            im[n] = np_in[n]
        in_maps.append(im)
    return in_maps


def run_on_device(inputs):
    nc = _get_nc()
    in_maps = _make_in_maps(inputs)
    res = bass_utils.run_bass_kernel_spmd(nc, in_maps, core_ids=list(range(NCORES)),
                                          trace=False)
    outs = [res.results[c]['out'] for c in range(NCORES)]
    return np.concatenate(outs, axis=0).astype(np.float32)


def kernel(**inputs):
    return run_on_device(inputs)
